# revision 15
# baseline (speedup 1.0000x reference)
"""Trainium2 8-core kernel for biased-attention with sigmoid gating.

Reference computation (per batch b):
  q = heads(q_x @ Wq) * C**-0.5 ; k = heads(kv_x @ Wk) ; v = heads(kv_x @ Wv)
  a = softmax(q k^T + bias1 + bias2, axis=-1)
  o = (a @ v) gated by sigmoid(q_x @ Wg + bg), then @ Wo + bo

Shapes: B=2, Q=K=2048, CQ=CK=CV=256, H=8, C=32, CO=256.

Sharding: 8 cores = 2 batches x 4 query-quarters (512 rows each). Each core
computes all 8 heads for its rows; no cross-core communication is needed.

Design (vs the 221us baseline):
  * exp(bias1+bias2) is folded on the HOST into one bf16 tensor EB
    (exp(s+b) = exp(s)*EB), cutting bias HBM traffic 4x.
  * per 128k x 1024(2 heads x 512q) tile: PE QK^T (row-tiled pair, bases
    0/32) -> ACT exp straight out of a 2-bank PSUM group -> DVE multiply by
    EB (bf16 2x mode; GpSimd must stay idle or it steals DVE's SBUF ports)
    -> PE PV matmuls (col-tiled pair, output bases 0/64).
  * V carries an all-ones column per head so PV emits softmax denominators
    for free; a small PE back-transpose restores natural orientation for
    the normalization, gating, and output projection.
  * projections are software-pipelined into the main loop (V and K chunks
    stream just ahead of their first use) and setup DMAs ride the scalar
    ring with merged weight/bias tensors so the exp pipeline starts early.
"""

import numpy as np

B, Q, K, CQ, H, C, CO = 2, 2048, 2048, 256, 8, 32, 256
HC = H * C  # 256
QS = Q // 4  # 512 query rows per core
NP = H // 2  # head pairs
KT_N = K // 128  # 16 k-tiles
N_CORES = 8
SCALE = float(C) ** -0.5

_CACHED = {}


def _build():
    import concourse.bass as bass
    import concourse.mybir as mybir
    import concourse.tile as tile
    from concourse import bacc
    from concourse.masks import make_identity

    f32 = mybir.dt.float32
    bf16 = mybir.dt.bfloat16
    AF = mybir.ActivationFunctionType
    ALU = mybir.AluOpType

    nc = bacc.Bacc(None, target_bir_lowering=False)

    # activations arrive host-transposed and pre-cast to bf16: [C, rows]
    qxTd = nc.declare_dram_parameter("qxT", [CQ, QS], bf16, isOutput=False)
    kvxTd = nc.declare_dram_parameter("kvxT", [CQ, K], bf16, isOutput=False)
    # EB = exp(bias1+bias2), host-transposed to [pair, k, j, q]
    ebd = nc.declare_dram_parameter("eb", [NP, K, 2, QS], bf16, isOutput=False)
    # weights concatenated on host: [CQ, 5*HC] = Wq*scale | Wk | Wg | Wv | Wo
    Wall = nc.declare_dram_parameter("Wall", [CQ, 5 * HC], bf16, isOutput=False)
    bgbo = nc.declare_dram_parameter("bgbo", [HC + CO], f32, isOutput=False)
    out = nc.declare_dram_parameter("out", [QS, CO], f32, isOutput=True)

    W_IDX = {"Wq": 0, "Wk": 1, "Wg": 2, "Wv": 3, "Wo": 4}

    with tile.TileContext(nc) as tc:
        with (
            tc.tile_pool(name="singles", bufs=1) as singles,
            tc.tile_pool(name="stage", bufs=3) as stage,
            tc.tile_pool(name="ebp", bufs=4) as ebp,
            tc.tile_pool(name="work", bufs=3) as work,
            tc.tile_pool(name="ework", bufs=4) as ework,
            tc.tile_pool(name="ps", bufs=1, space="PSUM") as psp,
        ):
            ident = singles.tile([128, 128], bf16)
            make_identity(nc, ident)

            # ---- setup loads, all on the scalar ring (configs overlap the
            # SP preamble; EB streams ride the sync ring) ----
            qxT = singles.tile([128, 2, QS], bf16, tag="qxT")
            nc.sync.dma_start(
                out=qxT, in_=qxTd[:, :].rearrange("(a p) q -> p a q", p=128)
            )
            wall = singles.tile([128, 2, 5 * HC], bf16, tag="wall")
            nc.sync.dma_start(
                out=wall, in_=Wall[:, :].rearrange("(a p) c -> p a c", p=128)
            )
            bgbo_bc = singles.tile([128, HC + CO], f32, tag="bgbo")
            nc.sync.dma_start(out=bgbo_bc, in_=bgbo[:].partition_broadcast(128))
            kvxT = singles.tile([128, 2, K], bf16, tag="kvxT")
            for kc in range(4):
                ksl = slice(kc * 512, (kc + 1) * 512)
                nc.sync.dma_start(
                    out=kvxT[:, :, ksl],
                    in_=kvxTd[:, ksl].rearrange("(a p) k -> p a k", p=128),
                )

            def wslc(name, ck, cols=None):
                base = W_IDX[name] * HC
                cols = cols if cols is not None else slice(0, HC)
                return wall[:, ck, base + cols.start:base + cols.stop]

            bg_bc = bgbo_bc[:, 0:HC]
            bo_bc = bgbo_bc[:, HC:HC + CO]
            tinyi = singles.tile([1, 2], f32, tag="tinyi")
            nc.vector.memset(tinyi, 0.0)
            tinyo = singles.tile([1, 2], bf16, tag="tinyo")
            nc.scalar.activation(tinyo, tinyi, AF.Exp)  # load the exp table

            # Heads packed two per pair slot at partition bases 0/32; K/Q
            # projections fill two pair slots per matmul (pairs 2a / 2a+1 at
            # bands 0/32 and 64/96).
            QT = singles.tile([128, 2, QS], bf16, tag="QT")
            KT = singles.tile([128, 2, K], bf16, tag="KT")

            for a in range(2):  # Q projection: needs only qxT + Wq
                ps = psp.tile([128, 2, QS], f32, tag="scores", bufs=2, name="qps")
                for ck in range(2):
                    nc.tensor.matmul(
                        ps[:, 0, :],
                        wslc("Wq", ck, slice(a * 128, (a + 1) * 128)),
                        qxT[:, ck, :],
                        start=(ck == 0),
                        stop=(ck == 1),
                    )
                nc.vector.tensor_copy(QT[:, a, :], ps[:, 0, :])

            # G = sigmoid(qx @ Wg + bg) early: Sigmoid and Exp activation
            # tables both load during the prologue.
            Gn = singles.tile([128, 4, HC], f32, tag="Gn")
            gt = singles.tile([128, 4, HC], f32, tag="gtmp")
            for qt in range(4):
                ps = psp.tile([128, 2, QS], f32, tag="scores", bufs=2, name="gps")
                for ck in range(2):
                    nc.tensor.matmul(
                        ps[:, 0, :HC],
                        qxT[:, ck, qt * 128:(qt + 1) * 128],
                        wslc("Wg", ck),
                        start=(ck == 0),
                        stop=(ck == 1),
                    )
                nc.vector.tensor_add(gt[:, qt, :], ps[:, 0, :HC], bg_bc)
            th = singles.tile([128, 4, HC], f32, tag="th")
            nc.scalar.activation(th, gt, AF.Tanh, scale=0.5)
            nc.vector.tensor_scalar(Gn, th, 0.5, 0.5, ALU.mult, ALU.add)

            def k_proj(a, kc):
                # fill KT slot a (heads 4a..4a+3), k-chunk kc (512 wide)
                ps = psp.tile([128, 2, QS], f32, tag="scores", bufs=2,
                              name="kps")
                for ck in range(2):
                    nc.tensor.matmul(
                        ps[:, 0, :],
                        wslc("Wk", ck, slice(a * 128, (a + 1) * 128)),
                        kvxT[:, ck, kc * 512:(kc + 1) * 512],
                        start=(ck == 0),
                        stop=(ck == 1),
                    )
                nc.vector.tensor_copy(
                    KT[:, a, kc * 512:(kc + 1) * 512], ps[:, 0, :]
                )

            k_proj(0, 0)

            # V natural [128k, 16kt, 8h, 33] bf16 with an all-ones column per
            # head (softmax denominators fall out of the PV matmul); per-kt
            # projection is interleaved into pair 0's loop.
            Vn = singles.tile([128, KT_N, H, 33], bf16, tag="Vn")
            nc.vector.memset(Vn[:, :, :, 32:33], 1.0)

            def v_proj(kt):
                vtile = psp.tile([128, 8, 32], f32, tag="scores", bufs=2,
                                 name="vps")
                for ck in range(2):
                    nc.tensor.matmul(
                        vtile[:, :, :],
                        kvxT[:, ck, kt * 128:(kt + 1) * 128],
                        wslc("Wv", ck),
                        start=(ck == 0),
                        stop=(ck == 1),
                    )
                nc.vector.tensor_copy(Vn[:, kt, :, 0:32], vtile[:, :, :])

            O_all = singles.tile([128, 4, HC], f32, tag="O_all")
            ogt_all = singles.tile([128, 2, 4, 128], bf16, tag="ogt_all")

            def gate_qt(hcc, qt):
                # gate + transpose heads [4*hcc, 4*hcc+4) for one quarter
                hs = slice(hcc * 128, (hcc + 1) * 128)
                og = stage.tile([128, 128], bf16, tag="og", name="og")
                nc.vector.tensor_tensor(
                    og, O_all[:, qt, hs], Gn[:, qt, hs], ALU.mult
                )
                ogt_ps = psp.tile([128, 128], bf16, tag="tr", bufs=2,
                                  name="ogt_ps")
                nc.tensor.transpose(ogt_ps, og, ident)
                nc.vector.tensor_copy(ogt_all[:, hcc, qt, :], ogt_ps)

            # ---- main attention loops (transposed orientation) ----
            for p in range(NP):
                a, b_ = p // 2, (p % 2) * 64  # KT/QT slot and partition base
                o_ps = psp.tile([128, QS, 1], f32, tag="o_acc", bufs=2)

                def emit_pv(kt, et):
                    for j in range(2):
                        h = 2 * p + j
                        nc.tensor.matmul(
                            o_ps[j * 64:j * 64 + 33, :, 0],
                            Vn[:, kt, h, :],
                            et[:, j * QS:(j + 1) * QS],
                            start=(kt == 0),
                            stop=(kt == KT_N - 1),
                        )

                pend = None  # deferred PV: PE emits QK(kt) before PV(kt-1)
                EBq = None
                for kt in range(KT_N):
                    if kt % 4 == 0:
                        q4 = kt // 4
                        # stream projections/gating just ahead of first use
                        if p == 0 and q4 < 3:
                            k_proj(0, q4 + 1)
                        elif p == 1:
                            k_proj(1, q4)
                        elif p == 2:
                            gate_qt(0, q4)
                        EBq = ebp.tile([128, 4, 2 * QS], bf16, tag="eb",
                                       bufs=4, name="EBq")
                        rows = slice(q4 * 512, (q4 + 1) * 512)
                        nc.sync.dma_start(
                            out=EBq,
                            in_=ebd[p, rows, :, :].rearrange(
                                "(s pp) j q -> pp s (j q)", pp=128
                            ),
                        )
                    if p == 0:
                        v_proj(kt)
                    ksl = slice(kt * 128, (kt + 1) * 128)
                    s_ps = psp.tile([128, 2 * QS], f32, tag="scores", bufs=2)
                    for j in range(2):
                        hb = b_ + j * 32
                        nc.tensor.matmul(
                            s_ps[:, j * QS:(j + 1) * QS],
                            KT[hb:hb + 32, a, ksl],
                            QT[hb:hb + 32, a, :],
                            start=True,
                            stop=True,
                            tile_position=(hb, 0),
                        )
                    es = ework.tile([128, 2 * QS], bf16, tag="es", bufs=4)
                    nc.scalar.activation(es, s_ps, AF.Exp)
                    et = ework.tile([128, 2 * QS], bf16, tag="et", bufs=4)
                    nc.vector.tensor_tensor(et, es, EBq[:, kt % 4, :], ALU.mult)
                    if pend is not None:
                        emit_pv(*pend)
                    pend = (kt, et)
                emit_pv(*pend)

                # epilogue per pair: copy o^T out of PSUM, back-transpose to
                # natural [q, c], then normalize by the ones-column sums.
                oT = []
                for j in range(2):
                    oTj = work.tile([33, QS], bf16, tag=f"oT{j}", name="oTj")
                    nc.vector.tensor_copy(oTj, o_ps[j * 64:j * 64 + 33, :, 0])
                    oT.append(oTj)
                on_ps = psp.tile([128, 4, 2, 34], bf16, tag="tr", bufs=2)
                for qt in range(4):
                    for j in range(2):
                        nc.tensor.transpose(
                            on_ps[:, qt, j, 0:33],
                            oT[j][:, qt * 128:(qt + 1) * 128],
                            ident[:33, :33],
                        )
                rinv = work.tile([128, 4, 2], f32, tag="rinv")
                nc.vector.reciprocal(rinv, on_ps[:, :, :, 32])
                for qt in range(4):
                    for j in range(2):
                        hcol = (2 * p + j) * 32
                        nc.vector.tensor_scalar_mul(
                            O_all[:, qt, hcol:hcol + 32],
                            on_ps[:, qt, j, 0:32],
                            rinv[:, qt, j:j + 1],
                        )

            # ---- gating half 1 + output projection ----
            for qt in range(4):
                gate_qt(1, qt)
            for qt in range(4):
                f_ps = psp.tile([128, 2, QS], f32, tag="scores", bufs=2,
                                name="fps")
                for hcc in range(2):
                    nc.tensor.matmul(
                        f_ps[:, 0, :CO],
                        ogt_all[:, hcc, qt, :],
                        wslc("Wo", hcc),
                        start=(hcc == 0),
                        stop=(hcc == 1),
                    )
                o_sb = stage.tile([128, CO], f32, tag="o_out", name="o_sb")
                nc.vector.tensor_add(o_sb, f_ps[:, 0, :CO], bo_bc)
                nc.sync.dma_start(out=out[qt * 128:(qt + 1) * 128, :], in_=o_sb)

    nc.compile()
    return nc


def _get_nc():
    if "nc" not in _CACHED:
        _CACHED["nc"] = _build()
    return _CACHED["nc"]


def kernel(**inputs):
    from concourse.bass_utils import run_bass_kernel_spmd

    import ml_dtypes

    bf = ml_dtypes.bfloat16
    nc = _get_nc()
    inp = {k: np.asarray(v, dtype=np.float32) for k, v in inputs.items()}
    wall = np.concatenate(
        [inp["Wq"] * SCALE, inp["Wk"], inp["Wg"], inp["Wv"], inp["Wo"]], axis=1
    ).astype(bf)
    bgbo = np.concatenate([inp["bg"], inp["bo"]]).astype(np.float32)
    # EB = exp(bias1 + bias2) in bf16; per-core layout [pair, k, j, q]
    ebt = inp["bias1"] + inp["bias2"]
    np.exp(ebt, out=ebt)
    ebf = ebt.astype(bf)  # [B, H, Q, K]
    del ebt
    in_maps = []
    for c in range(N_CORES):
        b, qi = c // 4, c % 4
        q0 = qi * QS
        x = ebf[b, :, q0:q0 + QS, :].reshape(NP, 2, QS, K)
        in_maps.append({
            "qxT": np.ascontiguousarray(inp["q_x"][b, q0:q0 + QS, :].T).astype(bf),
            "kvxT": np.ascontiguousarray(inp["kv_x"][b].T).astype(bf),
            "eb": np.ascontiguousarray(x.transpose(0, 3, 1, 2)),
            "Wall": wall,
            "bgbo": bgbo,
        })
    res = run_bass_kernel_spmd(nc, in_maps, core_ids=list(range(N_CORES)))
    outa = np.empty((B, Q, CO), np.float32)
    for c in range(N_CORES):
        b, qi = c // 4, c % 4
        outa[b, qi * QS:(qi + 1) * QS, :] = res.results[c]["out"]
    return outa
